# revision 1
# baseline (speedup 1.0000x reference)
"""Trainium2 Bass kernel for nn_Attention_19971597927194 (GNN message passing).

Destination-sharded, zero-collective design:
  - Edges sorted by destination i0; each of 8 cores owns 12500 consecutive
    destination nodes and every edge pointing into them. No cross-core
    reduction is needed.
  - Host does layout-only prep (concat / sort / index packing; no arithmetic
    on tensor values).
  - Destinations are packed into blocks of <=128 dests. Per block, edges are
    gathered via dma_gather (SWDGE MoE primitive): the per-edge source rows
    come from a [k|eigs|1|v] table (1280B rows) split in 4 chunks of 25000
    rows (int16 index limit), the per-edge destination rows from a local
    [q|eigs] table (768B rows). Scores, exp-weights, and a fused
    is_equal*weight one-hot mask (tensor_scalar) feed PE matmuls that
    accumulate [den|num] per destination in PSUM. Softmax normalization
    commutes with the segment sum, so it is applied once per destination at
    the end. Output rows are scattered with an indirect DMA.
"""
import sys

sys.path.insert(0, '/opt/trn_rl_repo')

import numpy as np

N_NODES = 100000
N_EDGES = 1000000
HID = 128
EDIM = 32
N_PATH = 6
NCORES = 8
P = 128

RU = 192                  # U row: [q(128) | eigs(32) | pad(32)] = 768B
RG = 320                  # G row: [k(128) | eigs(32) | 1 | v(128) | pad(31)] = 1280B
ONES_COL = HID + EDIM     # 160: the constant-1 column in G
NCHUNK = 4                # G table split (int16 index limit)
TC = 3                    # tiles per chunk group per block
T = NCHUNK * TC           # 12 tiles (of 128 slots) per block
CAPC = TC * P             # 384: per-chunk slot capacity per block
BB = 2                    # blocks per gather batch
D_CORE = N_NODES // NCORES
CHUNK = N_NODES // NCHUNK  # 25000

_INVSQRT = float(1.0 / np.sqrt(np.float32(HID)))


def _wrap_idx(lst):
    """dma_gather index layout: [128, n/16] = wrapped [16, n/16], tiled x8."""
    n = lst.shape[0]
    assert n % 16 == 0
    w = lst.reshape(n // 16, 16).T
    return np.tile(w, (8, 1)).astype(np.int16)


def _prepare(q, k, v, eigs, lambda0, path_emb, indices, path_type):
    q = np.asarray(q, dtype=np.float32)
    k = np.asarray(k, dtype=np.float32)
    v = np.asarray(v, dtype=np.float32)
    eigs = np.asarray(eigs, dtype=np.float32)
    lam0 = np.asarray(lambda0, dtype=np.float32).reshape(1, 1)
    pemb = np.asarray(path_emb, dtype=np.float32).reshape(1, N_PATH)
    i0 = np.asarray(indices[0])
    i1 = np.asarray(indices[1])
    pt = np.asarray(path_type)

    Gt = np.zeros((N_NODES, RG), dtype=np.float32)
    Gt[:, 0:HID] = k
    Gt[:, HID:HID + EDIM] = eigs
    Gt[:, ONES_COL] = 1.0
    Gt[:, ONES_COL + 1:ONES_COL + 1 + HID] = v
    Ut = np.zeros((N_NODES, RU), dtype=np.float32)
    Ut[:, 0:HID] = q
    Ut[:, HID:HID + EDIM] = eigs

    order = np.argsort(i0, kind='stable')
    i0s = i0[order].astype(np.int64)
    i1s = i1[order].astype(np.int64)
    pts = pt[order].astype(np.int64)
    core_bounds = np.searchsorted(i0s, np.arange(NCORES + 1) * D_CORE)
    eye = np.eye(N_PATH, dtype=np.float32)

    # ---- per-core block packing (consecutive dests; per-chunk slot quota) ----
    core_blocks = []
    for c in range(NCORES):
        lo, hi = core_bounds[c], core_bounds[c + 1]
        i0l = i0s[lo:hi] - c * D_CORE
        chl = (i1s[lo:hi] // CHUNK).astype(np.int64)
        deg_pc = np.zeros((D_CORE, NCHUNK), np.int64)
        np.add.at(deg_pc, (i0l, chl), 1)
        assert deg_pc.max() <= CAPC
        blocks = []
        ds = 0
        while ds < D_CORE:
            de = ds
            cnt = np.zeros(NCHUNK, np.int64)
            while de < D_CORE and de - ds < P and np.all(cnt + deg_pc[de] <= CAPC):
                cnt += deg_pc[de]
                de += 1
            assert de > ds
            blocks.append((ds, de))
            ds = de
        core_blocks.append((lo, hi, blocks))
    B = max(len(b) for _, _, b in core_blocks)
    if B % BB:
        B += BB - (B % BB)
    NBATCH = B // BB

    per_core = []
    for c in range(NCORES):
        lo, hi, blocks = core_blocks[c]
        i0l = i0s[lo:hi] - c * D_CORE
        i1c = i1s[lo:hi]
        ptc = pts[lo:hi]
        csum = np.concatenate([[0], np.cumsum(np.bincount(i0l, minlength=D_CORE))])

        gi16 = np.zeros((NBATCH, NCHUNK, 128, BB * TC * P // 16), dtype=np.int16)
        ui16 = np.zeros((NBATCH, 128, BB * T * P // 16), dtype=np.int16)
        dloc = np.full((B, P, T), 1.0e9, dtype=np.float32)
        oh = np.zeros((B, P, T, N_PATH), dtype=np.float32)
        scat = np.full((B, P), 1 << 20, dtype=np.int32)

        gl_lists = np.zeros((NBATCH, NCHUNK, BB * TC * P), dtype=np.int64)
        ul_lists = np.zeros((NBATCH, BB * T * P), dtype=np.int64)

        for b, (ds, de) in enumerate(blocks):
            bt, bb = divmod(b, BB)
            e0, e1 = csum[ds], csum[de]
            sl = slice(e0, e1)
            ii0 = i0l[sl]
            ii1 = i1c[sl]
            ipt = ptc[sl]
            ch = ii1 // CHUNK
            # order edges by chunk, stable
            o2 = np.argsort(ch, kind='stable')
            ii0, ii1, ipt, ch = ii0[o2], ii1[o2], ipt[o2], ch[o2]
            cc = np.concatenate([[0], np.cumsum(np.bincount(ch, minlength=NCHUNK))])
            for cch in range(NCHUNK):
                g0, g1 = cc[cch], cc[cch + 1]
                n_g = g1 - g0
                assert n_g <= CAPC
                j = np.arange(n_g)
                kk = j // P          # tile within chunk group (0..TC-1)
                pp = j % P
                # block-tile index t = cch*TC + kk ; slot in per-chunk idx
                # list for this batch: bb*CAPC + j
                dloc[b, pp, cch * TC + kk] = (ii0[g0:g1] - ds).astype(np.float32)
                oh[b, pp, cch * TC + kk, :] = eye[ipt[g0:g1]]
                gl_lists[bt, cch, bb * CAPC + j] = ii1[g0:g1] - cch * CHUNK
                # u index follows staging layout: staging tile s = cch*(BB*TC)
                # + bb*TC + kk, u-list position s*P + pp
                s_t = cch * (BB * TC) + bb * TC + kk
                ul_lists[bt, s_t * P + pp] = ii0[g0:g1]
            scat[b, 0:de - ds] = np.arange(ds, de, dtype=np.int32)

        for bt in range(NBATCH):
            for cch in range(NCHUNK):
                gi16[bt, cch] = _wrap_idx(gl_lists[bt, cch])
            ui16[bt] = _wrap_idx(ul_lists[bt])

        per_core.append(dict(
            G=Gt, U=Ut[c * D_CORE:(c + 1) * D_CORE],
            gi16=gi16.reshape(NBATCH, NCHUNK * 128, BB * TC * P // 16),
            ui16=ui16,
            dloc=dloc, oh=oh.reshape(B, P, T * N_PATH),
            scat=scat.transpose(1, 0).copy(),
            lam0=lam0, pemb=pemb,
        ))
    return per_core, B


def _build_bass(B):
    import concourse.bass as bass
    import concourse.bacc as bacc
    import concourse.mybir as mybir
    from concourse.tile import TileContext

    dt = mybir.dt
    Alu = mybir.AluOpType
    Act = mybir.ActivationFunctionType
    NBATCH = B // BB
    NIG = BB * TC * P          # idxs per chunk gather call (768)
    NIU = BB * T * P           # idxs per U gather call (3072)

    nc = bacc.Bacc(None)
    G = nc.declare_dram_parameter("G", [N_NODES, RG], dt.float32, isOutput=False)
    U = nc.declare_dram_parameter("U", [D_CORE, RU], dt.float32, isOutput=False)
    gi16 = nc.declare_dram_parameter("gi16", [NBATCH, NCHUNK * 128, NIG // 16],
                                     dt.int16, isOutput=False)
    ui16 = nc.declare_dram_parameter("ui16", [NBATCH, 128, NIU // 16],
                                     dt.int16, isOutput=False)
    dloc = nc.declare_dram_parameter("dloc", [B, P, T], dt.float32, isOutput=False)
    oh = nc.declare_dram_parameter("oh", [B, P, T * N_PATH], dt.float32,
                                   isOutput=False)
    scat = nc.declare_dram_parameter("scat", [P, B], dt.int32, isOutput=False)
    lam0 = nc.declare_dram_parameter("lam0", [1, 1], dt.float32, isOutput=False)
    pemb = nc.declare_dram_parameter("pemb", [1, N_PATH], dt.float32, isOutput=False)
    out = nc.declare_dram_parameter("out", [D_CORE, HID], dt.float32, isOutput=True)

    with TileContext(nc) as tc:
        with tc.tile_pool(name="const", bufs=1) as cpool, \
             tc.tile_pool(name="gath", bufs=2) as gpool, \
             tc.tile_pool(name="work", bufs=2) as wpool, \
             tc.tile_pool(name="small", bufs=3) as spool, \
             tc.tile_pool(name="psum", bufs=2, space="PSUM") as pspool, \
             tc.tile_pool(name="psc", bufs=1, space="PSUM") as pscpool:

            iota = cpool.tile([P, P], dt.float32)
            nc.gpsimd.iota(iota[:], pattern=[[1, P]], base=0, channel_multiplier=0,
                           allow_small_or_imprecise_dtypes=True)
            ones = cpool.tile([1, P], dt.float32)
            nc.vector.memset(ones[:], 1.0)
            scat_all = cpool.tile([P, B], dt.int32)
            nc.sync.dma_start(out=scat_all[:], in_=scat[:])

            tl = cpool.tile([1, 1], dt.float32)
            nc.sync.dma_start(out=tl[:], in_=lam0[:])
            tle = cpool.tile([1, 1], dt.float32)
            nc.scalar.activation(out=tle[:], in_=tl[:], func=Act.Exp)
            lps = pscpool.tile([P, 1], dt.float32)
            nc.tensor.matmul(out=lps[:], lhsT=ones[:], rhs=tle[:], start=True,
                             stop=True)
            lamb = cpool.tile([P, 1], dt.float32)
            nc.vector.tensor_copy(out=lamb[:], in_=lps[:])

            tp = cpool.tile([1, N_PATH], dt.float32)
            nc.sync.dma_start(out=tp[:], in_=pemb[:])
            tpe = cpool.tile([1, N_PATH], dt.float32)
            nc.scalar.activation(out=tpe[:], in_=tp[:], func=Act.Exp)
            nc.vector.tensor_scalar(out=tpe[:], in0=tpe[:], scalar1=5.0,
                                    scalar2=None, op0=Alu.min)
            pps = pscpool.tile([P, N_PATH], dt.float32)
            nc.tensor.matmul(out=pps[:], lhsT=ones[:], rhs=tpe[:], start=True,
                             stop=True)
            w1tab = cpool.tile([P, N_PATH], dt.float32)
            nc.vector.tensor_copy(out=w1tab[:], in_=pps[:])
            w1ap = w1tab[:]
            w1bc = bass.AP(w1ap.tensor, w1ap.offset,
                           [w1ap.ap[0], [0, T], w1ap.ap[1]])

            for bt in range(NBATCH):
                # ---- gathers for this batch (BB blocks) ----
                Gg = gpool.tile([P, BB * T, RG], dt.float32, tag="Gg")
                Ug = gpool.tile([P, BB * T, RU], dt.float32, tag="Ug")
                uit = spool.tile([P, NIU // 16], dt.int16, tag="uit")
                nc.sync.dma_start(out=uit[:], in_=ui16[bt])
                nc.gpsimd.dma_gather(Ug[:], U[:], uit[:], NIU, NIU, RU,
                                     single_packet=False)
                for cch in range(NCHUNK):
                    git = spool.tile([P, NIG // 16], dt.int16, tag=f"git{cch}")
                    nc.sync.dma_start(
                        out=git[:],
                        in_=gi16[bt, cch * 128:(cch + 1) * 128, :])
                    dst = Gg[:, cch * (BB * TC):(cch + 1) * (BB * TC), :]
                    nc.gpsimd.dma_gather(dst, G[cch * CHUNK:(cch + 1) * CHUNK, :],
                                         git[:], NIG, NIG, RG,
                                         single_packet=False)

                for bb in range(BB):
                    b = bt * BB + bb
                    # staging tile for block-tile t=(cch*TC+kk):
                    #   s(t) = cch*(BB*TC) + bb*TC + kk
                    def gsl(t):
                        cch, kk = divmod(t, TC)
                        return cch * (BB * TC) + bb * TC + kk

                    dlc = spool.tile([P, T], dt.float32, tag="dlc")
                    nc.sync.dma_start(out=dlc[:], in_=dloc[b])
                    oht = spool.tile([P, T, N_PATH], dt.float32, tag="oht")
                    nc.sync.dma_start(out=oht[:], in_=oh[b])

                    # 4-D views of this block's staging tiles:
                    # [P, cch(4), kk(TC), r]
                    Gap = Gg[:]
                    Gv = bass.AP(Gap.tensor, Gap.offset + bb * TC * RG,
                                 [Gap.ap[0], [BB * TC * RG, NCHUNK],
                                  [RG, TC], [1, RG]])
                    Uap = Ug[:]
                    Uv = bass.AP(Uap.tensor, Uap.offset + bb * TC * RU,
                                 [Uap.ap[0], [BB * TC * RU, NCHUNK],
                                  [RU, TC], [1, RU]])

                    RS = HID + EDIM  # 160: real score width
                    prod = wpool.tile([P, T, RS], dt.float32, tag="prod")
                    prod4 = prod[:].rearrange("p (c k) r -> p c k r", c=NCHUNK)
                    nc.vector.tensor_tensor(out=prod4, in0=Uv[:, :, :, 0:RS],
                                            in1=Gv[:, :, :, 0:RS], op=Alu.mult)
                    red1 = spool.tile([P, T], dt.float32, tag="red1")
                    nc.vector.tensor_reduce(
                        out=red1[:], in_=prod[:, :, 0:HID],
                        axis=mybir.AxisListType.X, op=Alu.add)
                    red2 = spool.tile([P, T], dt.float32, tag="red2")
                    nc.vector.tensor_reduce(
                        out=red2[:], in_=prod[:, :, HID:HID + EDIM],
                        axis=mybir.AxisListType.X, op=Alu.add)
                    score = spool.tile([P, T], dt.float32, tag="score")
                    nc.vector.tensor_scalar(out=score[:], in0=red1[:],
                                            scalar1=_INVSQRT, scalar2=None,
                                            op0=Alu.mult)
                    nc.vector.scalar_tensor_tensor(out=score[:], in0=red2[:],
                                                   scalar=lamb[:], in1=score[:],
                                                   op0=Alu.mult, op1=Alu.add)
                    nc.vector.tensor_scalar(out=score[:], in0=score[:],
                                            scalar1=2.0, scalar2=None,
                                            op0=Alu.min)
                    w0 = spool.tile([P, T], dt.float32, tag="w0")
                    nc.scalar.activation(out=w0[:], in_=score[:], func=Act.Exp)
                    nc.vector.tensor_scalar(out=w0[:], in0=w0[:], scalar1=5.0,
                                            scalar2=None, op0=Alu.min)
                    ohw = wpool.tile([P, T, N_PATH], dt.float32, tag="ohw")
                    nc.vector.tensor_tensor(out=ohw[:], in0=oht[:], in1=w1bc,
                                            op=Alu.mult)
                    w1 = spool.tile([P, T], dt.float32, tag="w1")
                    nc.vector.tensor_reduce(out=w1[:], in_=ohw[:],
                                            axis=mybir.AxisListType.X,
                                            op=Alu.add)

                    mw0 = wpool.tile([P, T, P], dt.float32, tag="mw0")
                    mw1 = wpool.tile([P, T, P], dt.float32, tag="mw1")
                    for t in range(T):
                        nc.vector.tensor_scalar(out=mw0[:, t, :], in0=iota[:],
                                                scalar1=dlc[:, t:t + 1],
                                                scalar2=w0[:, t:t + 1],
                                                op0=Alu.is_equal, op1=Alu.mult)
                        nc.vector.tensor_scalar(out=mw1[:, t, :], in0=iota[:],
                                                scalar1=dlc[:, t:t + 1],
                                                scalar2=w1[:, t:t + 1],
                                                op0=Alu.is_equal, op1=Alu.mult)
                    ps0 = pspool.tile([P, 1 + HID], dt.float32, tag="ps0")
                    ps1 = pspool.tile([P, 1 + HID], dt.float32, tag="ps1")
                    for t in range(T):
                        rhs = Gv[:, t // TC, t % TC, ONES_COL:ONES_COL + 1 + HID]
                        nc.tensor.matmul(out=ps0[:], lhsT=mw0[:, t, :], rhs=rhs,
                                         start=(t == 0), stop=(t == T - 1))
                    for t in range(T):
                        rhs = Gv[:, t // TC, t % TC, ONES_COL:ONES_COL + 1 + HID]
                        nc.tensor.matmul(out=ps1[:], lhsT=mw1[:, t, :], rhs=rhs,
                                         start=(t == 0), stop=(t == T - 1))

                    obuf = wpool.tile([P, HID], dt.float32, tag="obuf")
                    o1 = wpool.tile([P, HID], dt.float32, tag="o1")
                    for ps, dest in ((ps0, obuf), (ps1, o1)):
                        dz = spool.tile([P, 1], dt.float32, tag="dz")
                        nc.vector.tensor_scalar(out=dz[:], in0=ps[:, 0:1],
                                                scalar1=0.0, scalar2=None,
                                                op0=Alu.is_equal)
                        nc.vector.tensor_tensor(out=dz[:], in0=dz[:],
                                                in1=ps[:, 0:1], op=Alu.add)
                        nc.vector.tensor_scalar(out=dz[:], in0=dz[:], scalar1=2.0,
                                                scalar2=None, op0=Alu.mult)
                        rcp = spool.tile([P, 1], dt.float32, tag="rcp")
                        nc.vector.reciprocal(rcp[:], dz[:])
                        nc.scalar.activation(out=dest[:], in_=ps[:, 1:1 + HID],
                                             func=Act.Copy, scale=rcp[:])
                    nc.vector.tensor_tensor(out=obuf[:], in0=obuf[:], in1=o1[:],
                                            op=Alu.add)
                    nc.gpsimd.indirect_dma_start(
                        out=out[:],
                        out_offset=bass.IndirectOffsetOnAxis(
                            ap=scat_all[:, b:b + 1], axis=0),
                        in_=obuf[:], in_offset=None,
                        bounds_check=D_CORE - 1, oob_is_err=False)

    nc.finalize()
    return nc


_CACHE = {}


def _get_nc(B):
    if B not in _CACHE:
        _CACHE[B] = _build_bass(B)
    return _CACHE[B]


def run(inputs, trace=False):
    from concourse.bass_utils import run_bass_kernel_spmd
    per_core, B = _prepare(**inputs)
    nc = _get_nc(B)
    res = run_bass_kernel_spmd(nc, per_core, list(range(NCORES)), trace=trace)
    outs = [np.asarray(res.results[c]["out"]) for c in range(NCORES)]
    full = np.concatenate(outs, axis=0)
    return full, res


def kernel(**inputs):
    full, _ = run(inputs, trace=False)
    return full



# revision 7
# speedup vs baseline: 2.7801x; 2.7801x over previous
"""Trainium2 Bass kernel for nn_Attention_19971597927194 (GNN message passing).

Destination-sharded, input-minimized design:
  - Edges sorted by destination i0; each of 8 cores owns 12500 consecutive
    destination nodes and every edge pointing into them. No cross-core
    reduction of the output is needed.
  - The [k|eigs|1|v] source-node table arrives SHARDED (12500 rows per core,
    fp16, 640B rows) and is AllGathered on-device into a DRAM scratch table;
    per-edge source rows are then fetched locally with dma_gather (4 chunks
    of 25000 rows for the int16 index limit).
  - Destination-side q|eigs rows are block-materialized by the host
    ([B,128,160] fp16) and replicated to edge slots on-device with a one-hot
    PE matmul (no per-edge U gather, no per-edge q/eigs input bytes).
  - Per-edge scores, exp-weights and one-hot masks in fp16; the per-block
    [den|num] accumulation for both softmax channels runs as one-hot PE
    matmuls into PSUM; normalization commutes with the segment sum and is
    applied once per destination. Output rows (fp16) are scattered with an
    indirect DMA.
  - Host does layout-only prep (sort / pack / cast; no arithmetic on tensor
    values).
"""
import sys

sys.path.insert(0, '/opt/trn_rl_repo')

import numpy as np

N_NODES = 100000
N_EDGES = 1000000
HID = 128
EDIM = 32
N_PATH = 6
NCORES = 8
P = 128

RG = 384                  # kev row: [k(128)|eigs(32)|1|v(128)|pad(95)] fp16 = 768B
QE = 160                  # dest row: [q(128)|eigs(32)] fp16
ONES_COL = HID + EDIM     # 160: the constant-1 column in the kev row
NCHUNK = 4                # gather-table split (int16 index limit)
TC = 3                    # tiles per chunk group per block
T = NCHUNK * TC           # 12 tiles (of 128 slots) per block
CAPC = TC * P             # 384: per-chunk slot capacity per block
BB = 2                    # blocks per gather batch
D_CORE = N_NODES // NCORES
CHUNK = N_NODES // NCHUNK  # 25000
PAD_D = 30000.0           # dlc sentinel for pad slots (never equals 0..127)

_INVSQRT = float(1.0 / np.sqrt(np.float32(HID)))


def _wrap_idx(lst):
    """dma_gather index layout: [128, n/16] = wrapped [16, n/16], tiled x8."""
    n = lst.shape[0]
    assert n % 16 == 0
    w = lst.reshape(n // 16, 16).T
    return np.tile(w, (8, 1)).astype(np.int16)


def _prepare(q, k, v, eigs, lambda0, path_emb, indices, path_type):
    q = np.asarray(q, dtype=np.float32)
    k = np.asarray(k, dtype=np.float32)
    v = np.asarray(v, dtype=np.float32)
    eigs = np.asarray(eigs, dtype=np.float32)
    lam0 = np.asarray(lambda0, dtype=np.float32).reshape(1, 1)
    pemb = np.asarray(path_emb, dtype=np.float32).reshape(1, N_PATH)
    i0 = np.asarray(indices[0]).astype(np.int64)
    i1 = np.asarray(indices[1]).astype(np.int64)
    pt = np.asarray(path_type).astype(np.int64)

    # full fp16 source table [k|e|1|v|pad] (cast is layout-only prep)
    Gt = np.zeros((N_NODES, RG), dtype=np.float16)
    Gt[:, 0:HID] = k.astype(np.float16)
    Gt[:, HID:HID + EDIM] = eigs.astype(np.float16)
    Gt[:, ONES_COL] = np.float16(1.0)
    Gt[:, ONES_COL + 1:ONES_COL + 1 + HID] = v.astype(np.float16)
    # dest-side [q|e] fp16
    Qt = np.zeros((N_NODES, QE), dtype=np.float16)
    Qt[:, 0:HID] = q.astype(np.float16)
    Qt[:, HID:QE] = eigs.astype(np.float16)

    order = np.argsort(i0, kind='stable')
    i0s = i0[order]
    i1s = i1[order]
    pts = pt[order]
    core_bounds = np.searchsorted(i0s, np.arange(NCORES + 1) * D_CORE)

    # ---- per-core block packing (consecutive dests; per-chunk slot quota) ----
    core_blocks = []
    for c in range(NCORES):
        lo, hi = core_bounds[c], core_bounds[c + 1]
        i0l = i0s[lo:hi] - c * D_CORE
        chl = (i1s[lo:hi] // CHUNK).astype(np.int64)
        deg_pc = np.zeros((D_CORE, NCHUNK), np.int64)
        np.add.at(deg_pc, (i0l, chl), 1)
        assert deg_pc.max() <= CAPC
        blocks = []
        ds = 0
        while ds < D_CORE:
            de = ds
            cnt = np.zeros(NCHUNK, np.int64)
            while de < D_CORE and de - ds < P and np.all(cnt + deg_pc[de] <= CAPC):
                cnt += deg_pc[de]
                de += 1
            assert de > ds
            blocks.append((ds, de))
            ds = de
        core_blocks.append((lo, hi, blocks))
    B = max(len(b) for _, _, b in core_blocks)
    if B % BB:
        B += BB - (B % BB)
    NBATCH = B // BB

    per_core = []
    for c in range(NCORES):
        lo, hi, blocks = core_blocks[c]
        i0l = i0s[lo:hi] - c * D_CORE
        i1c = i1s[lo:hi]
        ptc_ = pts[lo:hi]
        csum = np.concatenate([[0], np.cumsum(np.bincount(i0l, minlength=D_CORE))])

        gi16 = np.zeros((NBATCH, NCHUNK, 128, BB * TC * P // 16), dtype=np.int16)
        dlc = np.full((B, P, T), PAD_D, dtype=np.float16)
        dlr = np.full((B, 1, T * P), PAD_D, dtype=np.float16)
        ptc = np.zeros((B, P, T), dtype=np.float16)
        qeB = np.zeros((B, P, QE), dtype=np.float16)
        scat = np.full((B, P), 1 << 20, dtype=np.int32)

        gl_lists = np.zeros((NBATCH, NCHUNK, BB * TC * P), dtype=np.int64)

        for b, (ds, de) in enumerate(blocks):
            bt, bb = divmod(b, BB)
            e0, e1 = csum[ds], csum[de]
            sl = slice(e0, e1)
            ii0 = i0l[sl]
            ii1 = i1c[sl]
            ipt = ptc_[sl]
            ch = ii1 // CHUNK
            # order edges by chunk, stable
            o2 = np.argsort(ch, kind='stable')
            ii0, ii1, ipt, ch = ii0[o2], ii1[o2], ipt[o2], ch[o2]
            cc = np.concatenate([[0], np.cumsum(np.bincount(ch, minlength=NCHUNK))])
            for cch in range(NCHUNK):
                g0, g1 = cc[cch], cc[cch + 1]
                n_g = g1 - g0
                assert n_g <= CAPC
                j = np.arange(n_g)
                kk = j // P          # tile within chunk group (0..TC-1)
                pp = j % P
                t = cch * TC + kk    # block-tile index
                dloc_v = (ii0[g0:g1] - ds).astype(np.float16)
                dlc[b, pp, t] = dloc_v
                dlr[b, 0, t * P + pp] = dloc_v
                ptc[b, pp, t] = ipt[g0:g1].astype(np.float16)
                # per-chunk idx list position for this batch: bb*CAPC + j
                gl_lists[bt, cch, bb * CAPC + j] = ii1[g0:g1] - cch * CHUNK
            qeB[b, 0:de - ds, :] = Qt[c * D_CORE + ds:c * D_CORE + de]
            scat[b, 0:de - ds] = np.arange(ds, de, dtype=np.int32)

        for bt in range(NBATCH):
            for cch in range(NCHUNK):
                gi16[bt, cch] = _wrap_idx(gl_lists[bt, cch])

        per_core.append(dict(
            GS=np.ascontiguousarray(Gt[c * D_CORE:(c + 1) * D_CORE]),
            qeB=qeB,
            gi16=gi16.reshape(NBATCH, NCHUNK * 128, BB * TC * P // 16),
            dlc=dlc, dlr=dlr, ptcb=ptc,
            scat=scat.transpose(1, 0).copy(),
            lam0=lam0, pemb=pemb,
        ))
    return per_core, B


def _build_bass(B):
    import concourse.bass as bass
    import concourse.bacc as bacc
    import concourse.mybir as mybir
    from concourse.tile import TileContext

    dt = mybir.dt
    Alu = mybir.AluOpType
    Act = mybir.ActivationFunctionType
    NBATCH = B // BB
    NIG = BB * TC * P          # idxs per chunk gather call (768)

    nc = bacc.Bacc(None, num_devices=NCORES)
    GS = nc.declare_dram_parameter("GS", [D_CORE, RG], dt.float16, isOutput=False)
    qeB = nc.declare_dram_parameter("qeB", [B, P, QE], dt.float16, isOutput=False)
    gi16 = nc.declare_dram_parameter("gi16", [NBATCH, NCHUNK * 128, NIG // 16],
                                     dt.int16, isOutput=False)
    dlc = nc.declare_dram_parameter("dlc", [B, P, T], dt.float16, isOutput=False)
    dlr = nc.declare_dram_parameter("dlr", [B, 1, T * P], dt.float16,
                                    isOutput=False)
    ptcb = nc.declare_dram_parameter("ptcb", [B, P, T], dt.float16,
                                     isOutput=False)
    scat = nc.declare_dram_parameter("scat", [P, B], dt.int32, isOutput=False)
    lam0 = nc.declare_dram_parameter("lam0", [1, 1], dt.float32, isOutput=False)
    pemb = nc.declare_dram_parameter("pemb", [1, N_PATH], dt.float32,
                                     isOutput=False)
    out = nc.declare_dram_parameter("out", [D_CORE, HID], dt.float16,
                                    isOutput=True)

    with TileContext(nc) as tc:
        with tc.tile_pool(name="dram", bufs=1, space="DRAM") as dpool, \
             tc.tile_pool(name="const", bufs=1) as cpool, \
             tc.tile_pool(name="gath", bufs=2) as gpool, \
             tc.tile_pool(name="work", bufs=2) as wpool, \
             tc.tile_pool(name="small", bufs=3) as spool, \
             tc.tile_pool(name="psA", bufs=1, space="PSUM") as psA, \
             tc.tile_pool(name="psB", bufs=1, space="PSUM") as psB, \
             tc.tile_pool(name="psC", bufs=1, space="PSUM") as psC:

            # ---- allgather the kev table shard into a full DRAM table ----
            gsb = dpool.tile([D_CORE, RG], dt.float16)
            Gfull = dpool.tile([N_NODES, RG], dt.float16)
            nc.gpsimd.dma_start(gsb[:], GS[:])
            nc.gpsimd.collective_compute(
                "AllGather", Alu.bypass,
                replica_groups=[list(range(NCORES))],
                ins=[gsb[:].opt()],
                outs=[Gfull[:].opt()],
            )

            # ---- constants ----
            iota16 = cpool.tile([P, P], dt.float16)
            nc.gpsimd.iota(iota16[:], pattern=[[1, P]], base=0,
                           channel_multiplier=0,
                           allow_small_or_imprecise_dtypes=True)
            iotac = cpool.tile([P, 1], dt.float32)
            nc.gpsimd.iota(iotac[:], pattern=[[1, 1]], base=0,
                           channel_multiplier=1,
                           allow_small_or_imprecise_dtypes=True)
            ones32 = cpool.tile([1, P], dt.float32)
            nc.vector.memset(ones32[:], 1.0)
            ones16 = cpool.tile([1, P], dt.float16)
            nc.vector.memset(ones16[:], 1.0)
            scat_all = cpool.tile([P, B], dt.int32)
            nc.sync.dma_start(out=scat_all[:], in_=scat[:])

            # exp(lambda0), replicated across partitions; fp16 copy
            tl = cpool.tile([1, 1], dt.float32)
            nc.sync.dma_start(out=tl[:], in_=lam0[:])
            tle = cpool.tile([1, 1], dt.float32)
            nc.scalar.activation(out=tle[:], in_=tl[:], func=Act.Exp)
            pre = psA.tile([P, T * P], dt.float32, tag="drep")
            nc.tensor.matmul(out=pre[:, 0:1], lhsT=ones32[:], rhs=tle[:],
                             start=True, stop=True)
            lamb32 = cpool.tile([P, 1], dt.float32)
            nc.vector.tensor_copy(out=lamb32[:], in_=pre[:, 0:1])

            # w1 table: min(exp(pemb), 5), replicated; fp16
            tp = cpool.tile([1, N_PATH], dt.float32)
            nc.sync.dma_start(out=tp[:], in_=pemb[:])
            tpe = cpool.tile([1, N_PATH], dt.float32)
            nc.scalar.activation(out=tpe[:], in_=tp[:], func=Act.Exp)
            nc.vector.tensor_scalar(out=tpe[:], in0=tpe[:], scalar1=5.0,
                                    scalar2=None, op0=Alu.min)
            nc.tensor.matmul(out=pre[:, 1:1 + N_PATH], lhsT=ones32[:],
                             rhs=tpe[:], start=True, stop=True)
            w1rep = cpool.tile([P, N_PATH], dt.float32)
            nc.vector.tensor_copy(out=w1rep[:], in_=pre[:, 1:1 + N_PATH])

            for bt in range(NBATCH):
                # ---- source-row gathers for this batch (BB blocks) ----
                Gg = gpool.tile([P, BB * T, RG], dt.float16, tag="Gg")
                for cch in range(NCHUNK):
                    git = spool.tile([P, NIG // 16], dt.int16, tag=f"git{cch}")
                    nc.sync.dma_start(
                        out=git[:],
                        in_=gi16[bt, cch * 128:(cch + 1) * 128, :])
                    dst = Gg[:, cch * (BB * TC):(cch + 1) * (BB * TC), :]
                    nc.gpsimd.dma_gather(dst,
                                         Gfull[cch * CHUNK:(cch + 1) * CHUNK, :],
                                         git[:], NIG, NIG, RG,
                                         single_packet=False)

                for bb in range(BB):
                    b = bt * BB + bb

                    # block-tile t=(cch*TC+kk) lives at staging tile
                    #   s(t) = cch*(BB*TC) + bb*TC + kk
                    # 4-D view of this block's staging: [P, cch, kk, RG]
                    Gap = Gg[:]
                    Gv = bass.AP(Gap.tensor, Gap.offset + bb * TC * RG,
                                 [Gap.ap[0], [BB * TC * RG, NCHUNK],
                                  [RG, TC], [1, RG]])

                    # ---- per-block small inputs ----
                    dlc_t = spool.tile([P, T], dt.float16, tag="dlc")
                    nc.sync.dma_start(out=dlc_t[:], in_=dlc[b])
                    dlr_t = spool.tile([1, T * P], dt.float16, tag="dlr")
                    nc.sync.dma_start(out=dlr_t[:], in_=dlr[b])
                    ptc_t = spool.tile([P, T], dt.float16, tag="ptc")
                    nc.sync.dma_start(out=ptc_t[:], in_=ptcb[b])
                    qe_t = spool.tile([P, QE], dt.float16, tag="qe")
                    nc.sync.dma_start(out=qe_t[:], in_=qeB[b])

                    # scaled dest rows: q/sqrt(d), eigs*exp(lambda0)
                    qes = spool.tile([P, QE], dt.float16, tag="qes")
                    nc.vector.tensor_scalar(out=qes[:, 0:HID],
                                            in0=qe_t[:, 0:HID],
                                            scalar1=_INVSQRT, scalar2=None,
                                            op0=Alu.mult)
                    nc.vector.tensor_scalar(out=qes[:, HID:QE],
                                            in0=qe_t[:, HID:QE],
                                            scalar1=lamb32[:, 0:1],
                                            scalar2=None, op0=Alu.mult)

                    # ---- replicate dlc across partitions: drep[d, slot] ----
                    drep = psA.tile([P, T * P], dt.float32, tag="drep")
                    for g in range(3):
                        nc.tensor.matmul(out=drep[:, g * 512:(g + 1) * 512],
                                         lhsT=ones16[:],
                                         rhs=dlr_t[:, g * 512:(g + 1) * 512],
                                         start=True, stop=True)
                    onehot = wpool.tile([P, T * P], dt.float16, tag="onehot")
                    nc.vector.tensor_scalar(out=onehot[:], in0=drep[:],
                                            scalar1=iotac[:, 0:1], scalar2=None,
                                            op0=Alu.is_equal)

                    # ---- per-edge dest rows: qrep[slot, 160] = onehot^T @ qes
                    # 3 tiles per 512-fp32 PSUM bank so no matmul output
                    # crosses a bank boundary: tile t at col 512*(t//3)+160*(t%3)
                    qrep = psB.tile([P, 2048], dt.float32, tag="qrep")
                    qcol = lambda t: 512 * (t // 3) + QE * (t % 3)
                    for t in range(T):
                        nc.tensor.matmul(
                            out=qrep[:, qcol(t):qcol(t) + QE],
                            lhsT=onehot[:, t * P:(t + 1) * P],
                            rhs=qes[:], start=True, stop=True)
                    qrep_v = bass.AP(qrep[:].tensor, qrep[:].offset,
                                     [qrep[:].ap[0], [512, NCHUNK], [QE, TC],
                                      [1, QE]])
                    qrep16 = wpool.tile([P, T, QE], dt.float16, tag="qrep16")
                    nc.scalar.activation(
                        out=qrep16[:].rearrange("p (c k) r -> p c k r",
                                                c=NCHUNK),
                        in_=qrep_v, func=Act.Copy)

                    # ---- per-edge scores ----
                    prod = wpool.tile([P, T, QE], dt.float16, tag="prod")
                    prod4 = prod[:].rearrange("p (c k) r -> p c k r", c=NCHUNK)
                    nc.vector.tensor_tensor(out=prod4, in0=Gv[:, :, :, 0:QE],
                                            in1=qrep16[:].rearrange(
                                                "p (c k) r -> p c k r",
                                                c=NCHUNK),
                                            op=Alu.mult)
                    score = spool.tile([P, T], dt.float32, tag="score")
                    nc.vector.tensor_reduce(out=score[:], in_=prod[:],
                                            axis=mybir.AxisListType.X,
                                            op=Alu.add)
                    w0 = spool.tile([P, T], dt.float16, tag="w0")
                    nc.scalar.activation(out=w0[:], in_=score[:], func=Act.Exp)
                    nc.vector.tensor_scalar(out=w0[:], in0=w0[:], scalar1=5.0,
                                            scalar2=None, op0=Alu.min)

                    # w1[slot,t] = w1tab[ptc]
                    w1 = spool.tile([P, T], dt.float16, tag="w1")
                    tmp1 = spool.tile([P, T], dt.float16, tag="tmp1")
                    for j in range(N_PATH):
                        dst1 = w1 if j == 0 else tmp1
                        nc.vector.tensor_scalar(out=dst1[:], in0=ptc_t[:],
                                                scalar1=float(j),
                                                scalar2=w1rep[:, j:j + 1],
                                                op0=Alu.is_equal, op1=Alu.mult)
                        if j > 0:
                            nc.vector.tensor_tensor(out=w1[:], in0=w1[:],
                                                    in1=tmp1[:], op=Alu.add)

                    # ---- one-hot masks * weights ----
                    mask = wpool.tile([P, T, P], dt.float16, tag="mask")
                    dlc_b = bass.AP(dlc_t[:].tensor, dlc_t[:].offset,
                                    [dlc_t[:].ap[0], [1, T], [0, P]])
                    iota_b = bass.AP(iota16[:].tensor, iota16[:].offset,
                                     [iota16[:].ap[0], [0, T], [1, P]])
                    nc.vector.tensor_tensor(out=mask[:], in0=dlc_b, in1=iota_b,
                                            op=Alu.is_equal)
                    mw0 = wpool.tile([P, T, P], dt.float16, tag="mw0")
                    w0_b = bass.AP(w0[:].tensor, w0[:].offset,
                                   [w0[:].ap[0], [1, T], [0, P]])
                    nc.vector.tensor_tensor(out=mw0[:], in0=mask[:], in1=w0_b,
                                            op=Alu.mult)
                    mw1 = wpool.tile([P, T, P], dt.float16, tag="mw1")
                    w1_b = bass.AP(w1[:].tensor, w1[:].offset,
                                   [w1[:].ap[0], [1, T], [0, P]])
                    nc.vector.tensor_tensor(out=mw1[:], in0=mask[:], in1=w1_b,
                                            op=Alu.mult)

                    # ---- [den|num] accumulation for both channels ----
                    ps01 = psC.tile([P, 2 * (1 + HID)], dt.float32, tag="ps01")
                    for t in range(T):
                        rhs = Gv[:, t // TC, t % TC, ONES_COL:ONES_COL + 1 + HID]
                        nc.tensor.matmul(out=ps01[:, 0:1 + HID],
                                         lhsT=mw0[:, t, :], rhs=rhs,
                                         start=(t == 0), stop=(t == T - 1))
                    for t in range(T):
                        rhs = Gv[:, t // TC, t % TC, ONES_COL:ONES_COL + 1 + HID]
                        nc.tensor.matmul(out=ps01[:, 1 + HID:2 * (1 + HID)],
                                         lhsT=mw1[:, t, :], rhs=rhs,
                                         start=(t == 0), stop=(t == T - 1))

                    # ---- normalize + combine channels (x0.5 folded as 2*den)
                    obuf = spool.tile([P, HID], dt.float16, tag="obuf")
                    o1 = spool.tile([P, HID], dt.float16, tag="o1")
                    for ci, dest in ((0, obuf), (1, o1)):
                        den = ps01[:, ci * (1 + HID):ci * (1 + HID) + 1]
                        num = ps01[:, ci * (1 + HID) + 1:(ci + 1) * (1 + HID)]
                        dz = spool.tile([P, 1], dt.float32, tag=f"dz{ci}")
                        nc.vector.tensor_scalar(out=dz[:], in0=den, scalar1=0.0,
                                                scalar2=None, op0=Alu.is_equal)
                        nc.vector.tensor_tensor(out=dz[:], in0=dz[:], in1=den,
                                                op=Alu.add)
                        nc.vector.tensor_scalar(out=dz[:], in0=dz[:],
                                                scalar1=2.0, scalar2=None,
                                                op0=Alu.mult)
                        rcp = spool.tile([P, 1], dt.float32, tag=f"rcp{ci}")
                        nc.vector.reciprocal(rcp[:], dz[:])
                        nc.scalar.activation(out=dest[:], in_=num,
                                             func=Act.Copy, scale=rcp[:])
                    nc.vector.tensor_tensor(out=obuf[:], in0=obuf[:], in1=o1[:],
                                            op=Alu.add)
                    nc.gpsimd.indirect_dma_start(
                        out=out[:],
                        out_offset=bass.IndirectOffsetOnAxis(
                            ap=scat_all[:, b:b + 1], axis=0),
                        in_=obuf[:], in_offset=None,
                        bounds_check=D_CORE - 1, oob_is_err=False)

    nc.finalize()
    return nc


_CACHE = {}


def _get_nc(B):
    if B not in _CACHE:
        _CACHE[B] = _build_bass(B)
    return _CACHE[B]


def run(inputs, trace=False):
    from concourse.bass_utils import run_bass_kernel_spmd
    per_core, B = _prepare(**inputs)
    nc = _get_nc(B)
    res = run_bass_kernel_spmd(nc, per_core, list(range(NCORES)), trace=trace)
    outs = [np.asarray(res.results[c]["out"]) for c in range(NCORES)]
    full = np.concatenate(outs, axis=0).astype(np.float32)
    return full, res


def kernel(**inputs):
    full, _ = run(inputs, trace=False)
    return full
